# revision 6
# baseline (speedup 1.0000x reference)
"""Trainium2 Bass kernel for batched self-attention (dense_transformer).

Reference math (per batch b, with N = H*W = 4096 tokens):
    kq  = w_kq @ x + b_kq            [128, N]
    sim = kq^T @ kq                  [N, N]   (symmetric Gram matrix)
    attn = softmax(sim, axis=-1)
    ctx = attn @ v^T  (v = w_v @ x + b_v)
    out = w_o @ ctx + b_o

Sharding: data-parallel over batch, one batch per NeuronCore (B=8, 8 cores).

Device algorithm (transpose-free symmetric softmax):
  * b_v is folded into the output bias on the host (attention rows sum to 1,
    so  attn @ (v + b_v 1^T)^T = attn @ v_raw^T + 1 b_v^T).
  * E[m,n] = exp(sim[m,n] - ssq[n]) where ssq[n] = ||kq_n||^2 = sim[n,n].
    The per-column shift is injected with a pair of concurrent rank-1
    matmuls (ones x -ssq, row-tiled to PE row groups 0 and 32) in the
    same PSUM accumulation group as the Gram matmul, so exp needs no
    bias and never overflows (sim[m,n] <= sqrt(ssq_m ssq_n)).
    Per-column shifts cancel exactly in the softmax normalization.
  * The whole front end runs in bf16: x ships from the host as bf16, the
    kq/v projections are single bf16 matmuls, and the Gram logits are one
    bf16 matmul per block. ssq is computed from the *rounded* kqh tile
    (ssq[n] = sum_k kqh[k,n]^2), so E's diagonal is exp(0) = 1 exactly and
    all remaining Gram rounding cancels in the softmax ratio (the logits'
    diagonal dominates by ~+100 for this distribution, so off-diagonal
    perturbations are multiplied by ~e^-100).
  * E is computed in [m(part), n(free)] blocks which serve directly as the
    moving operand of the ctx matmul (contraction over m) - no transposes.
  * Z[n] = sum_m E[m,n] equals the row sums sum_n E[m,n] by symmetry, so it
    falls out of the ScalarE activation accumulator for free-axis sums.
  * The output projection computes out^T tiles [n(part), o(free)], where the
    1/Z[n] softmax normalization is a per-partition scalar multiply fused
    with the +bias add in one scalar_tensor_tensor op.
"""

import os
import tempfile

import numpy as np

# The libneuronxla NEFF cache keys on an HLO-module hash that does not cover
# the bass custom-call backend_config (where the actual kernel BIR lives), so
# a stale cache entry from a *different* kernel build with the same tensor
# signature silently substitutes the wrong NEFF. Two defenses: a private
# cache dir (honored when no boot hook pinned the cache singleton earlier),
# and a build-id nonce input whose shape makes this build's HLO hash unique.
os.environ.setdefault("NEURON_COMPILE_CACHE_URL",
                      tempfile.mkdtemp(prefix="neff-cache-"))
KERNEL_BUILD_ID = 213

_CACHE = {}

N_CORES = 8
C_IN = 256
CK = 128
CO = 256
N_TOK = 4096
PW = 1024  # panel width (exp batch), must divide N_TOK, multiple of 512


def _build_nc(n_tok=N_TOK, pw=PW, reps=1):
    """Build the kernel module. reps>1 repeats the whole per-batch kernel
    body (including input/output DMA) reps times inside one NEFF — used by
    test.py to measure sustained per-execution device time above the axon
    dispatch noise floor. The math is identical each rep."""
    import concourse.bacc as bacc
    import concourse.mybir as mybir
    import concourse.tile as tile
    from concourse.bass import ts

    dt = mybir.dt
    f32 = dt.float32
    f32r = dt.float32r
    bf16 = dt.bfloat16
    AF = mybir.ActivationFunctionType
    OP = mybir.AluOpType

    NT = n_tok // 128      # number of 128-token tiles
    NP = n_tok // pw       # number of panels
    HV = pw // 512         # 512-wide halves per panel

    nc = bacc.Bacc("TRN2", target_bir_lowering=False, debug=False,
                   num_devices=N_CORES)

    x_d = nc.dram_tensor("x", [C_IN, n_tok], bf16, kind="ExternalInput").ap()
    wkq_d = nc.dram_tensor("wkqT", [C_IN, CK], bf16, kind="ExternalInput").ap()
    wv_d = nc.dram_tensor("wvT", [C_IN, CK], bf16, kind="ExternalInput").ap()
    wo_d = nc.dram_tensor("woT", [CK, CO], bf16, kind="ExternalInput").ap()
    bkq_d = nc.dram_tensor("bkq", [CK, 1], f32, kind="ExternalInput").ap()
    # Unused input whose shape encodes the build id (and reps variant):
    # keeps this build's HLO module hash distinct from any previously cached
    # bass kernel with the same real tensor signature (see cache note at top
    # of file -- the cache does not hash the custom-call backend_config).
    nc.dram_tensor("nonce", [1, _nonce_width(reps)], f32,
                   kind="ExternalInput")
    out_d = nc.dram_tensor("outT", [n_tok, CO], bf16,
                       kind="ExternalOutput").ap()

    with tile.TileContext(nc) as tc:
        with tc.tile_pool(name="persist", bufs=1) as pp, \
             tc.tile_pool(name="epool", bufs=6) as ep:

            # ---------- persistent SBUF tiles ----------
            x0 = pp.tile([128, n_tok], bf16, tag="x0")
            x1 = pp.tile([128, n_tok], bf16, tag="x1")
            kqh = pp.tile([128, n_tok], bf16, tag="kqh")
            kq2 = pp.tile([128, n_tok], bf16, tag="kq2")
            vT = pp.tile([128, n_tok], bf16, tag="vT")     # col block i = vT of m-tile i
            ctx = pp.tile([128, n_tok], bf16, tag="ctx")   # [vc, n]
            negssq = pp.tile([33, n_tok], bf16, tag="negssq")
            wkq0 = pp.tile([128, CK], bf16, tag="wkq0")
            wkq1 = pp.tile([128, CK], bf16, tag="wkq1")
            wv0 = pp.tile([128, CK], bf16, tag="wv0")
            wv1 = pp.tile([128, CK], bf16, tag="wv1")
            wo = pp.tile([128, CO], bf16, tag="wo")
            bkq = pp.tile([128, 1], f32, tag="bkq")
            ones_rb = pp.tile([33, 128], bf16, tag="ones_rb")  # rank-1 lhsT
            mones_c = pp.tile([128, 1], bf16, tag="mones_c")   # -ssq lhsT (-1s)
            ones_fr = pp.tile([1, 128], f32, tag="ones_fr")
            mones_f = pp.tile([128, 1], f32, tag="mones_f")
            zparts = pp.tile([128, NT * NP], f32, tag="zparts")
            zred = pp.tile([128, NT], f32, tag="zred")
            obig = pp.tile([128, NT * CO], bf16, tag="obig")
            zrec = pp.tile([128, NT], f32, tag="zrec")

            for _rep in range(reps):
                _emit_body(nc, tc, mybir, tile, ts, ep, n_tok, pw, NT, NP, HV,
                           x_d, wkq_d, wv_d, wo_d, bkq_d, out_d,
                           x0, x1, kqh, kq2, vT, ctx, negssq, wkq0, wkq1,
                           wv0, wv1, wo, bkq, ones_rb, mones_c, ones_fr,
                           mones_f, zparts, zred, obig, zrec)

    nc.compile()
    return nc


def _emit_body(nc, tc, mybir, tile, ts, ep, n_tok, pw, NT, NP, HV,
               x_d, wkq_d, wv_d, wo_d, bkq_d, out_d,
               x0, x1, kqh, kq2, vT, ctx, negssq, wkq0, wkq1,
               wv0, wv1, wo, bkq, ones_rb, mones_c, ones_fr,
               mones_f, zparts, zred, obig, zrec):
    dt = mybir.dt
    f32 = dt.float32
    bf16 = dt.bfloat16
    AF = mybir.ActivationFunctionType
    OP = mybir.AluOpType

    # ---------- P0: loads ----------
    nc.sync.dma_start(wkq0[:], wkq_d[0:128, :])
    nc.sync.dma_start(wkq1[:], wkq_d[128:256, :])
    nc.sync.dma_start(bkq[:], bkq_d[:])
    for c in range(4):
        cs = slice(c * (n_tok // 4), (c + 1) * (n_tok // 4))
        nc.sync.dma_start(x0[:, cs], x_d[0:128, cs])
        nc.scalar.dma_start(x1[:, cs], x_d[128:256, cs])
    nc.scalar.dma_start(wv0[:], wv_d[0:128, :])
    nc.scalar.dma_start(wv1[:], wv_d[128:256, :])
    nc.scalar.dma_start(wo[:], wo_d[:])
    nc.vector.memset(mones_f[:], -1.0)
    nc.vector.memset(ones_fr[:], 1.0)
    nc.vector.tensor_copy(ones_rb[0:1, :], ones_fr[:])
    nc.vector.tensor_copy(ones_rb[32:33, :], ones_fr[:])
    nc.vector.tensor_copy(mones_c[:], mones_f[:])

    with tc.tile_pool(name="mpsum", bufs=4, space="PSUM") as mp:
        # ---------- P1a+P2 interleaved per 512-block: kqh = bf16(
        # w_kq @ x + b_kq), then negssq[n] = -sum_k kqh[k,n]^2.
        # Emitting the DVE square + PE column-sum right after each
        # block keeps every engine's pipe full during the preamble.
        for t in range(n_tok // 512):
            ps = mp.tile([128, 512], f32)
            nc.tensor.matmul(ps[:], wkq0[:], x0[:, ts(t, 512)],
                             start=True, stop=False)
            nc.tensor.matmul(ps[:], wkq1[:], x1[:, ts(t, 512)],
                             start=False, stop=True)
            nc.scalar.activation(kqh[:, ts(t, 512)], ps[:],
                                 AF.Identity, bias=bkq[:])
            nc.vector.tensor_mul(kq2[:, ts(t, 512)],
                                 kqh[:, ts(t, 512)],
                                 kqh[:, ts(t, 512)])
            ps2 = mp.tile([128, 512], f32)
            nc.tensor.matmul(ps2[0:1, :], mones_c[:],
                             kq2[:, ts(t, 512)],
                             start=True, stop=True)
            nc.vector.tensor_copy(negssq[0:1, ts(t, 512)], ps2[0:1, :])
            nc.sync.dma_start(negssq[32:33, ts(t, 512)],
                              negssq[0:1, ts(t, 512)])

        # ---------- P1b: vT tiles (no bias; folded into boe) ----------
        for i in range(NT):
            ps = mp.tile([128, 512], f32)
            nc.tensor.matmul(ps[:, 0:128], x0[:, ts(i, 128)], wv0[:],
                             start=True, stop=False)
            nc.tensor.matmul(ps[:, 0:128], x1[:, ts(i, 128)], wv1[:],
                             start=False, stop=True)
            nc.vector.tensor_copy(vT[:, ts(i, 128)], ps[:, 0:128])

    # ---------- P3: main attention loop ----------
    # Software-pipelined with a SKEW-iteration lookahead: the ctx matmuls
    # for tile i are emitted AFTER the Gram matmuls for tile i+SKEW, so
    # the ScalarE exp latency for tile i hides behind later Grams and the
    # PE instruction stream never stalls (stalls re-throttle the PE clock
    # to 1.2 GHz via the HAM activity monitor).
    def pe_ctx(prev):
        e_p, i_p, ctxps_p, _ = prev
        for h in range(HV):
            sl = slice(h * 512, h * 512 + 512)
            nc.tensor.matmul(ctxps_p[:, sl], vT[:, ts(i_p, 128)],
                             e_p[:, sl],
                             start=(i_p == 0), stop=(i_p == NT - 1))

    SKEW = 4  # iterations of lookahead between exp and its ctx use

    with tc.tile_pool(name="spsum", bufs=2, space="PSUM") as sp, \
         tc.tile_pool(name="cpsum", bufs=2, space="PSUM") as cp:
        pending = []

        def drain_one():
            prev = pending.pop(0)
            pe_ctx(prev)
            if prev[1] == NT - 1:  # last tile of its panel
                jj = prev[3]
                nc.vector.tensor_copy(ctx[:, ts(jj, pw)], prev[2][:])

        for j in range(NP):
            ctxps = cp.tile([128, pw], f32)
            for i in range(NT):
                sps = sp.tile([128, pw], f32)
                if j == 0 and i == 0:
                    # PE warm-up: dependency-free back-to-back matmuls
                    # into the first sps tile (overwritten by the
                    # start=True rank-1 below). The HAM clock gate only
                    # un-throttles the PE to 2.4 GHz after observing a
                    # fully-busy 3413 ns activity window; the main
                    # loop's micro-gaps make that almost impossible, so
                    # without this burst the whole kernel runs at
                    # 1.2 GHz. It must sit AFTER the PSUM pool-open
                    # barrier, whose PE-idle window re-throttles the
                    # clock.
                    for _w in range(4):
                        nc.tensor.matmul(sps[:, 0:512], wkq0[:],
                                         x0[:, 0:512],
                                         start=True, stop=True)
                # Gram blocks (one kqh_i weight load): kqh_i^T kqh
                for h in range(HV):
                    sl = slice(h * 512, h * 512 + 512)
                    nsl = slice(j * pw + h * 512,
                                j * pw + h * 512 + 512)
                    nc.tensor.matmul(sps[:, sl], kqh[:, ts(i, 128)],
                                     kqh[:, nsl],
                                     start=True, stop=False)
                # rank-1 additions of -ssq[n]: K=1 occupies one
                # 32-row group, so the two halves run CONCURRENTLY
                # in the PE array: h=0 on rows 0-31, h=1 (operands
                # at base partition 32) on rows 32-63.
                for h in range(HV):
                    sl = slice(h * 512, h * 512 + 512)
                    nsl = slice(j * pw + h * 512,
                                j * pw + h * 512 + 512)
                    rp = 32 * h
                    nc.tensor.matmul(sps[:, sl],
                                     ones_rb[rp:rp + 1, :],
                                     negssq[rp:rp + 1, nsl],
                                     start=False, stop=True)
                if len(pending) >= SKEW:
                    drain_one()
                e = ep.tile([128, pw], bf16)
                zslot = zparts[:, i * NP + j: i * NP + j + 1]
                # Z row-sums: mostly on DVE (ScalarE exp is the critical
                # engine in steady state; accum_out costs +187ns there),
                # with a small ScalarE share to keep DVE just under it.
                if i % 16 == 0:
                    nc.scalar.activation(e[:], sps[:], AF.Exp,
                                         accum_out=zslot)
                else:
                    nc.scalar.activation(e[:], sps[:], AF.Exp)
                    nc.vector.tensor_reduce(
                        zslot, e[:], axis=mybir.AxisListType.X,
                        op=OP.add)
                pending.append((e, i, ctxps, j))
        while pending:
            drain_one()

    # ---------- P4: Z, output projection, normalize + bias ----------
    # The normalized tiles accumulate into one SBUF buffer (bf16)
    # and ship with 4 chunked DMAs: per-tile dma_starts cost ~600ns
    # each on the Sync engine and serialized the whole epilogue.
    zp3 = zparts[:].rearrange("p (i j) -> p i j", j=NP)
    nc.vector.tensor_reduce(zred[:], zp3, axis=mybir.AxisListType.X,
                            op=OP.add)
    nc.vector.reciprocal(zrec[:], zred[:])
    TPC = NT // 8  # tiles per output DMA chunk
    with tc.tile_pool(name="ppsum", bufs=4, space="PSUM") as prp:
        # re-warm the PE clock after the pool-open/zred barrier;
        # overwritten by the first out-projection matmul below
        wps2 = prp.tile([128, CO], f32)
        for _w in range(6):
            nc.tensor.matmul(wps2[:], ctx[:, 0:128], wo[:],
                             start=True, stop=True)
        for i in range(NT):
            ps = prp.tile([128, CO], f32)
            nc.tensor.matmul(ps[:], ctx[:, ts(i, 128)], wo[:],
                             start=True, stop=True)
            if i % 2 == 0:
                nc.scalar.activation(obig[:, ts(i, CO)], ps[:],
                                     AF.Identity,
                                     scale=zrec[:, i:i + 1])
            else:
                nc.vector.tensor_scalar_mul(obig[:, ts(i, CO)],
                                            ps[:],
                                            zrec[:, i:i + 1])
            if i % TPC == TPC - 1:
                c = i // TPC
                dst = out_d[c * TPC * 128:(c + 1) * TPC * 128, :]
                dst = dst.rearrange("(i p) o -> p i o", p=128)
                srcb = obig[:, c * TPC * CO:(c + 1) * TPC * CO]
                srcb = srcb.rearrange("p (i o) -> p i o", o=CO)
                nc.sync.dma_start(dst, srcb)


def _nonce_width(reps):
    return KERNEL_BUILD_ID * 16 + reps


def _get_nc():
    if "nc" not in _CACHE:
        _CACHE["nc"] = _build_nc()
    return _CACHE["nc"]


def _host_prep(x, w_kq, b_kq, w_v, b_v, w_o, b_o):
    import ml_dtypes
    bf = ml_dtypes.bfloat16
    B = x.shape[0]
    xf = np.ascontiguousarray(x.reshape(B, C_IN, N_TOK)).astype(bf)
    wkqT = np.ascontiguousarray(w_kq.T).astype(bf)
    wvT = np.ascontiguousarray(w_v.T).astype(bf)
    woT = np.ascontiguousarray(w_o.T).astype(bf)
    bkq2 = np.ascontiguousarray(b_kq.reshape(CK, 1)).astype(np.float32)
    boe = (w_o.astype(np.float64) @ b_v.astype(np.float64)
           + b_o.astype(np.float64)).astype(np.float32).reshape(CO, 1)
    return xf, wkqT, wvT, woT, bkq2, np.ascontiguousarray(boe)


def kernel(x, w_kq, b_kq, w_v, b_v, w_o, b_o):
    from concourse.bass_utils import run_bass_kernel_spmd

    x = np.asarray(x)
    B, C, H, W = x.shape
    xf, wkqT, wvT, woT, bkq2, boe = _host_prep(
        np.asarray(x), np.asarray(w_kq), np.asarray(b_kq), np.asarray(w_v),
        np.asarray(b_v), np.asarray(w_o), np.asarray(b_o))

    nc = _get_nc()
    nonce = np.zeros((1, _nonce_width(1)), dtype=np.float32)
    in_maps = [{
        "x": xf[b],
        "wkqT": wkqT,
        "wvT": wvT,
        "woT": woT,
        "bkq": bkq2,
        "nonce": nonce,
    } for b in range(B)]
    res = run_bass_kernel_spmd(nc, in_maps, core_ids=list(range(N_CORES)))
    out = np.empty((B, CO, H, W), dtype=np.float32)
    for b in range(B):
        out[b] = (res.results[b]["outT"].astype(np.float32).T
                  + boe).reshape(CO, H, W)
    return out

